# revision 36
# baseline (speedup 1.0000x reference)
"""CAP-memory loss kernel for Trainium2 (8 NeuronCores).

The only heavy part of the reference is
    sims = normalize(features) @ normalize(mem0.reshape(C*L, D)).T     [B, C*L]
which streams the full 256 MB proxy memory. The C*L axis is sharded across
the 8 cores (camera c -> core c, 4096 rows each); each core runs a
DMA/PE-balanced fp8(e4m3) DoubleRow matmul over its 8 MB shard and returns
its [B, 4096] block of raw dot products in fp16.

The fp8 result is used ONLY to select top-k candidates; every value that
enters the loss is recomputed exactly in f32 on the host:
  - per-camera CE logits: 8 x [32, 2048]x[2048, 4096] BLAS (~2 GFLOP),
    with the EMA-scatter columns patched from P = fn @ new_n.T (the
    scatter changes only B rows of the memory),
  - cross-camera positives and the BG_KNN hardest negatives: gathered and
    recomputed from a 256-candidate shortlist (fp8 ranking noise << the
    shortlist margin), so the final loss matches the f32 reference to
    ~1e-7 while the device stream is quarter-width.
"""

import numpy as np

C, L, D = 8, 4096, 2048
B = 256
BETA = 0.05
ALPHA = 0.01
CROSSCAM_EPOCH = 5
BG_KNN = 50
N_CORES = 8

_CACHE = {}


def _patch_tile_drain():
    """The walrus in this container rejects instructions with more than one
    sync wait; the stock TileContext exit puts every end-of-kernel wait on a
    single SP Drain. Spread them over dedicated single-wait nops instead."""
    import concourse.mybir as mybir
    import concourse.tile as tile
    from concourse.vector_clock import ScopedClock

    if getattr(tile.TileContext, "_drain_split_patch", False):
        return

    def _drain_and_barrier(self, tick_clock, wait_clock):
        nc = self.nc
        nop = nc.sync.nop(nofuse=True)
        wait_clock.add_sem_waits(
            nop.ins, ScopedClock({None: tick_clock.global_clock})
        )
        waits = list(nop.ins.sync_info.on_wait or [])
        if len(waits) > 1:
            nop.ins.sync_info = mybir.SyncInfo(on_wait=[waits[0]], on_update=[])
            for w in waits[1:]:
                extra = nc.sync.nop(nofuse=True)
                extra.ins.sync_info = mybir.SyncInfo(on_wait=[w], on_update=[])
        nc.sync.drain()
        nc.all_engine_barrier()
        assert self.sems is not None
        popped = nc._tile_sem_poison_stack.pop()
        assert popped is self._sem_poison
        nc.clear_and_free_semaphores(list(self.sems.allocated().values()))
        nc.all_engine_barrier()

    tile.TileContext._drain_and_barrier = _drain_and_barrier
    tile.TileContext._drain_split_patch = True


def _patch_tile_wait_split(max_waits=1):
    """This walrus rejects instructions carrying more than one sync wait.
    Before Tile lowers the scheduled instruction list, move extra waits onto
    same-engine NoOps inserted just before the offending instruction (engine
    queues are FIFO, so waiting earlier on the same engine is equivalent)."""
    import concourse.mybir as mybir
    import concourse.tile as tile

    if getattr(tile.TileContext, "_wait_split_patch", False):
        return
    orig = tile.TileContext._lower_ordered_insts
    counter = [0]

    def patched(self, ordered):
        for insts in ordered.values():
            new = []
            for inst in insts:
                try:
                    si = inst.sync_info
                    waits = list(si.on_wait or []) if si is not None else []
                except AttributeError:
                    waits = []
                if len(waits) > max_waits:
                    keep = waits[len(waits) - max_waits :]
                    for w in waits[: len(waits) - max_waits]:
                        counter[0] += 1
                        nop = mybir.InstNoOp(name=f"waitsplit-{counter[0]}")
                        nop.engine = inst.engine
                        nop.sync_info = mybir.SyncInfo(on_wait=[w], on_update=[])
                        new.append(nop)
                    inst.sync_info = mybir.SyncInfo(
                        on_wait=keep, on_update=list(si.on_update or [])
                    )
                new.append(inst)
            insts[:] = new
        return orig(self, ordered)

    tile.TileContext._lower_ordered_insts = patched
    tile.TileContext._wait_split_patch = True


def build_sims_program(
    Lsh=L, Dd=D, Bb=B, mm_dtype="float8e4", out_dtype="float16", double_row=True
):
    """Bass program: s0[i, r] = sum_d fnT[d, i] * mT[d, r] (un-normalized).

    double_row=True (fp8 only): contraction chunks are 256 logical rows held
    as [128 partitions, 2] pairs (virtual 128x256 PE array, 2 MACs/cell/cyc).
    Logical row d = chunk*KROW + j*128 + p for both operands; any consistent
    (p, j) -> d mapping is valid since the cell computes w0*m0 + w1*m1.

    Inputs  fnT  [128, KC*PJ*Bb]  (normalized features, chunked on host)
            mT   [Dd/PJ, PJ*Lsh]  (memory shard, chunked on host)
    Output  s0   [Bb, Lsh]        (raw dot products; host applies 1/row-norm)
    """
    import concourse.bass as bass
    import concourse.mybir as mybir
    import concourse.tile as tile

    _patch_tile_drain()
    _patch_tile_wait_split()
    dt = mybir.dt
    mmdt = getattr(dt, mm_dtype)
    outdt = getattr(dt, out_dtype)
    PJ = 2 if double_row else 1         # logical rows per partition element
    KROW = 128 * PJ
    perf_mode = mybir.MatmulPerfMode.DoubleRow if double_row else None

    assert Dd % KROW == 0 and Bb % 128 == 0 and Lsh % 512 == 0
    KC = Dd // KROW                     # contraction chunks
    NG = Bb // 128                      # output partition groups
    # PSUM holds NG * pass_width fp32 per partition (16 KB total)
    pass_width = min(Lsh, 4096 // NG // 512 * 512)
    NH = Lsh // pass_width              # output column passes
    RS = pass_width // 512              # 512-wide psum banks per pass

    nc = bass.Bass()
    fnT_d = nc.declare_dram_parameter(
        "fnT", [128, KC * PJ * Bb], mmdt, isOutput=False
    )
    mT_d = nc.declare_dram_parameter("mT", [Dd // PJ, PJ * Lsh], mmdt, isOutput=False)
    s0_d = nc.declare_dram_parameter("s0", [Bb, Lsh], outdt, isOutput=True)

    with tile.TileContext(nc) as tc:
        with (
            tc.tile_pool(name="const", bufs=1) as const_pool,
            tc.tile_pool(name="mt", bufs=8) as mt_pool,
            tc.tile_pool(name="out", bufs=2) as out_pool,
            tc.tile_pool(name="psum", bufs=1, space="PSUM") as psum_pool,
        ):
            fnT_sb = const_pool.tile([128, KC, PJ, Bb], mmdt, tag="fnT")
            nc.sync.dma_start(
                fnT_sb[:], fnT_d[:].rearrange("p (c j i) -> p c j i", c=KC, j=PJ)
            )

            # HAM warm-up: PE sits idle ~5us while the first tiles stream in,
            # so the first real matmuls would run at the 1.2 GHz cold clock.
            # A burst of zero matmuls during the fill keeps the activity
            # window busy and the real stream starts at 2.4 GHz.
            warm = const_pool.tile([128, PJ, 512], mmdt, tag="warm")
            nc.gpsimd.memset(warm[:], 0.0)
            wps = psum_pool.tile([128, 512], dt.float32, tag="ps0_0", name="warm_ps")
            for _ in range(8):
                nc.tensor.matmul(
                    wps[:],
                    warm[:, :, :128],
                    warm[:],
                    start=True,
                    stop=True,
                    perf_mode=perf_mode,
                )

            pending_out = []
            for h in range(NH):
                ps = {}
                for g in range(NG):
                    for rs in range(RS):
                        ps[g, rs] = psum_pool.tile(
                            [128, 512], dt.float32, tag=f"ps{g}_{rs}",
                            name=f"ps{g}_{rs}_{h}",
                        )
                outs = [
                    out_pool.tile(
                        [128, pass_width], outdt, tag=f"out{g}",
                        name=f"out{g}_{h}",
                    )
                    for g in range(NG)
                ]
                for k in range(KC):
                    mt = mt_pool.tile([128, PJ, pass_width], mmdt, tag="mt")
                    # host layout groups [h][j][r] per row, so this DMA reads
                    # one contiguous PJ*pass_width run per partition
                    nc.sync.dma_start(
                        mt[:],
                        mT_d[k * 128 : (k + 1) * 128, :].rearrange(
                            "p (h j r) -> p h j r", h=NH, j=PJ
                        )[:, h],
                    )

                    for g in range(NG):
                        for rs in range(RS):
                            if double_row:
                                lhsT = fnT_sb[:, k, :, g * 128 : (g + 1) * 128]
                                rhs = mt[:, :, rs * 512 : (rs + 1) * 512]
                            else:
                                lhsT = fnT_sb[:, k, 0, g * 128 : (g + 1) * 128]
                                rhs = mt[:, 0, rs * 512 : (rs + 1) * 512]
                            nc.tensor.matmul(
                                ps[g, rs][:],
                                lhsT,
                                rhs,
                                start=(k == 0),
                                stop=(k == KC - 1),
                                perf_mode=perf_mode,
                            )
                for g in range(NG):
                    for rs in range(RS):
                        # alternate evacuation between DVE and ACT so the
                        # serial copy chain at a pass boundary halves
                        if (g * RS + rs) % 2 == 0:
                            nc.vector.tensor_copy(
                                outs[g][:, rs * 512 : (rs + 1) * 512],
                                ps[g, rs][:],
                            )
                        else:
                            nc.scalar.copy(
                                outs[g][:, rs * 512 : (rs + 1) * 512],
                                ps[g, rs][:],
                            )
                    # fire each half of the group's output as soon as its
                    # two evacuation copies retire: overlaps the end-of-pass
                    # copy chain with the output transfer
                    for half in range(2):
                        hw2 = pass_width // 2

                        def fire(g=g, h=h, outs=outs, half=half, hw2=hw2):
                            nc.sync.dma_start(
                                s0_d[
                                    g * 128 : (g + 1) * 128,
                                    h * pass_width + half * hw2 : h * pass_width + (half + 1) * hw2,
                                ],
                                outs[g][:, half * hw2 : (half + 1) * hw2],
                            )
                        pending_out.append(fire)
            # all output DMAs at the end of the SP stream: they only wait on
            # already-retired evacuations, so the mt prefetch never stalls
            for fire in pending_out:
                fire()
    return nc


def _ensure_ntff_hook():
    """bass_utils' trace path imports antenv.axon_hooks, which this image's
    antenv lacks. Provide the module and register the ctypes NTFF hook the
    boot would have installed."""
    import sys
    import types

    try:
        import antenv.axon_hooks  # noqa: F401

        return
    except ImportError:
        pass
    import antenv

    mod = types.ModuleType("antenv.axon_hooks")
    state = {"h": None}
    mod.set_axon_ntff_profile_hook = lambda h: state.__setitem__("h", h)
    mod.get_axon_ntff_profile_hook = lambda: state["h"]
    sys.modules["antenv.axon_hooks"] = mod
    antenv.axon_hooks = mod
    try:
        from trn_agent_boot.trn_boot import _ntff_profile_via_ctypes

        h = _ntff_profile_via_ctypes("/opt/axon/libaxon_pjrt.so")
        if h is not None:
            mod.set_axon_ntff_profile_hook(h)
    except Exception:
        pass


def _get_program():
    if "nc" not in _CACHE:
        _CACHE["nc"] = build_sims_program()
    return _CACHE["nc"]


def _mm_np_dtype():
    import ml_dtypes

    return ml_dtypes.float8_e4m3


def _prep_mT(m, mmnp, n_pass=2):
    """[L, D] memory shard -> [D/2, 2*L] device layout: row (k*128+p) holds
    [h][j][r] so each (h, k) tile DMA is one contiguous run per partition;
    logical row d = k*256 + j*128 + p."""
    Lc, Dd = m.shape
    pw = Lc // n_pass
    return np.ascontiguousarray(
        m.T.reshape(Dd // 256, 2, 128, n_pass, pw)
        .transpose(0, 2, 3, 1, 4)
        .reshape(Dd // 2, 2 * Lc),
        dtype=mmnp,
    )


def _device_sims(fn, mem0, invn_full):
    """fn [B, D] normalized; mem0 [C, L, D]; invn_full [C*L] reciprocal row
    norms. Returns sims [B, C*L] (normalized), matmul on the 8 NeuronCores."""
    from concourse.bass_utils import run_bass_kernel_spmd

    nc = _get_program()
    Bb, Dd = fn.shape
    mmnp = _mm_np_dtype()
    # [D, B] -> [KC, 2, 128, B] -> [128, KC, 2, B] -> [128, KC*2*B]
    # (logical row d = c*256 + j*128 + p, pre-chunked for one linear DMA)
    fnT = np.ascontiguousarray(
        fn.T.reshape(Dd // 256, 2, 128, Bb).transpose(2, 0, 1, 3).reshape(128, -1),
        dtype=mmnp,
    )
    in_maps = []
    for c in range(N_CORES):
        in_maps.append({"fnT": fnT, "mT": _prep_mT(mem0[c], mmnp)})
    import os

    kwargs = {}
    if os.environ.get("KERNEL_TRACE"):
        _ensure_ntff_hook()
        kwargs = {"trace": True, "trace_cores": [0]}
    res = run_bass_kernel_spmd(nc, in_maps, core_ids=list(range(N_CORES)), **kwargs)
    _CACHE["exec_time_ns"] = res.exec_time_ns
    _CACHE["trace"] = res.instructions_and_trace
    s0 = np.concatenate(
        [res.results[c]["s0"].astype(np.float32) for c in range(N_CORES)], axis=1
    )
    return s0 * invn_full[None, :]


def _logsumexp(x, axis):
    m = np.max(x, axis=axis, keepdims=True)
    return m + np.log(np.sum(np.exp(x - m), axis=axis, keepdims=True))


def kernel(
    features,
    targets,
    cams,
    all_pseudo_label,
    all_img_cams,
    init_intra_id_feat,
    epoch,
    batch_ind,
):
    f = np.asarray(features, dtype=np.float32)
    targets = np.asarray(targets)
    cams = np.asarray(cams)
    mem0 = np.asarray(init_intra_id_feat, dtype=np.float32)   # [C, L, D]
    percam = B // C

    fn = f / np.linalg.norm(f, axis=1, keepdims=True)
    mflat = mem0.reshape(C * L, D)
    invn_full = 1.0 / np.sqrt(np.einsum("rd,rd->r", mflat, mflat))

    # --- heavy part on device: sims = fn @ normalize(mem0_flat).T ---
    sims = _device_sims(fn, mem0, invn_full)                  # [B, C*L]

    # --- EMA update (only its effect on the CE logits is needed) ---
    old = mem0[cams, targets]                                 # [B, D]
    new = ALPHA * old + (1.0 - ALPHA) * f
    new_n = new / np.linalg.norm(new, axis=1, keepdims=True)
    # memn rows get normalized once more in the reference; idempotent but
    # replicate for exactness of the patched columns
    new_n = new_n / np.linalg.norm(new_n, axis=1, keepdims=True)
    P = fn @ new_n.T                                          # [B, B]

    # --- per-camera proxy CE; the diagonal blocks are recomputed exactly on
    # host (2 GFLOP in BLAS), so the device result only drives top-k
    # candidate selection ---
    logits = np.empty((C, percam, L), dtype=np.float32)
    for c in range(C):
        blk = (
            fn[c * percam : (c + 1) * percam] @ mflat[c * L : (c + 1) * L].T
        ) * invn_full[None, c * L : (c + 1) * L]
        for j in np.nonzero(cams == c)[0]:                    # scatter order: last wins
            blk[:, targets[j]] = P[c * percam : (c + 1) * percam, j]
        logits[c] = blk
    logits /= BETA
    lsm = logits - _logsumexp(logits, axis=-1)
    t = targets.reshape(C, percam)
    ce = -np.take_along_axis(lsm, t[..., None], axis=-1)[..., 0]
    loss = ce.mean(axis=1).sum()

    # --- cross-camera associative loss ---
    # The device sims are fp16-precision; the entries that enter the loss
    # directly (positives + the BG_KNN hardest negatives) are recomputed
    # exactly on host from candidates selected with a safety margin.
    if int(epoch) >= CROSSCAM_EPOCH:
        CAND = 256
        pos = targets[:, None] + np.arange(C, dtype=np.int64)[None, :] * L
        rows = np.arange(B)[:, None]
        m_pos = mflat[pos.reshape(-1)].reshape(B, C, D)
        pos_sims = (
            np.matmul(m_pos, fn[:, :, None])[..., 0] * invn_full[pos]
        )                                                     # [B, C] exact
        masked = np.array(sims)
        masked[rows, pos] = -np.inf
        cand = np.argpartition(-masked, CAND - 1, axis=1)[:, :CAND]   # [B, CAND]
        m_c = mflat[cand.reshape(-1)].reshape(B, CAND, D)
        cvals = (
            np.matmul(m_c, fn[:, :, None])[..., 0] * invn_full[cand]
        )                                                     # [B, CAND] exact
        topv = -np.sort(-cvals, axis=1)[:, :BG_KNN]
        cat = np.concatenate([pos_sims / BETA, topv / BETA], axis=1).astype(
            np.float32
        )
        ls2 = cat - _logsumexp(cat, axis=1)
        per = -ls2[:, :C].sum(axis=1) / C
        loss = loss + 0.5 * per.reshape(C, percam).mean(axis=1).sum()

    return np.asarray([loss], dtype=np.float32)
